# revision 46
# baseline (speedup 1.0000x reference)
"""Trainium2 Bass kernel for CrossModalAttention (B=65536, DIM=768, 3 heads,
q_len=1, kv_len=3) — data-parallel over 8 NeuronCores.

Layout strategy ("layout T"): all on-chip activations are kept transposed,
[dim, rows], so every projection matmul contracts dim-chunks on the partition
axis with zero on-chip transposes.  The host passes pre-transposed activations
(query.T, key_s.T) and pre-transposed weights, and transposes the
[768, 8192] per-core outputs back at the end.

v3 changes (each verified against perfetto traces on hardware; baseline was
PE 88% busy at 1.371ms, this version measures 1.125ms):
  * V and O projections fused into one: since softmax weights sum to 1,
      out = sum_h (sum_s attn_{h,s} k_s) @ (Wo_h Wv_h)^T + (bo + Wo bv)
    The per-head key mix k~_h runs on DVE in bf16, the fused projection is
    108 matmuls instead of V(108)+O(36) — 36 fewer big PE matmuls/tile.
  * The fused projection + LayerNorm of iteration i-1 is emitted inside
    iteration i (after its K-proj/scores phase), so the PE never waits on
    the softmax -> broadcast -> key-mix DVE chain of the same iteration;
    the attn broadcasts are emitted after its first chain (mid_cb) so the
    ACT copies feeding the key mix are not stuck behind stage_b's ACT work.
    The last iteration emits its key mix right after the broadcasts (no
    next iteration to hide it under).
  * Elementwise traffic is bf16 in SBUF via wide [P, KO*r] tensor_tensor
    ops, with stride-0 broadcast APs for the per-row attn/rstd factors.
    Measured DVE modes on HW: tensor_tensor hits 2x with 16-bit packed
    operands; scalar_tensor_tensor runs at 1x (the cost model's 4x_2p for
    it does NOT materialize on silicon — do not "optimize" back to stt).
    GpSimd elementwise is ~1.3us per [128,512] op and contends with DVE
    for SBUF ports — only partition_broadcast runs there.
  * softmax reciprocal via reciprocal_approx_fast (~5x faster than
    InstReciprocal); its f32 output must NOT be bitcast to f32r for the
    matmul (BIR verifier: f32r consumers need f32r-rounded producers) —
    a tiny ACT bf16 copy feeds the sel9 matmul instead.
  * x (= out + residual) and y are bf16; the f32 residual copy of query is
    dropped (input DMA -1.5MB/tile) and the output DMA is bf16 (host casts).
  * Magic-seed rsqrt with a single Newton step (~1.8e-3 relative, enough
    for the 2e-2 gate); rstd/mu*rstd broadcast to 128 partitions on the
    otherwise-idle GpSimd (partition_broadcast needs partition-0 input).

Per 512-row tile (252 big + ~29 small matmuls, PE ~90% busy):
  Q/K projections      : PE bf16 matmuls, fp32 PSUM
  scores (QK dot/head) : wide DVE bf16 product over both head chunks +
                         in-place pair sum + PE ones-matmul partition
                         reduction into a [9, R] psum tile (selection
                         matmuls delayed 5 groups behind the products)
  softmax (S=3)        : ACT exp (scale=1/16) + PE selection-matmul sums +
                         DVE fast reciprocal; no max-subtraction needed
  key mix k~_h         : k2 + a_{h,0}(k0-k2) + a_{h,1}(k1-k2): attn rows
                         broadcast via PE rank-1 matmuls -> ACT bf16 copy
                         -> 4 wide DVE ops per head
  fused out-proj       : PE bf16 matmuls of k~ against (Wo_h Wv_h)^T
  residual + LayerNorm : PE ones-matmul partition sums for mean/var,
                         DVE magic-rsqrt + 1 Newton step, bf16 normalize
"""

import sys

if "/opt/trn_rl_repo" not in sys.path:
    sys.path.insert(0, "/opt/trn_rl_repo")

from contextlib import ExitStack

import numpy as np

import concourse.bass as bass
import concourse.bacc as bacc
import concourse.mybir as mybir
import concourse.tile as tile
from concourse.bass import AP
from concourse.bass_utils import run_bass_kernel_spmd

DIM = 768
P = 128
KO = DIM // P  # 6 chunks of the feature dim
H = 3          # heads
S = 3          # kv positions
HD = DIM // H  # 256 head dim
NCORES = 8
B = 65536
BCORE = B // NCORES
R = 512        # rows (batch elements) per tile iteration
EPS = 1e-5
MAGIC = 0x5F3759DF

AF = mybir.ActivationFunctionType
OP = mybir.AluOpType
F32 = mybir.dt.float32
F32R = mybir.dt.float32r
BF16 = mybir.dt.bfloat16
I32 = mybir.dt.int32


def _mm(nc, out, lhsT, rhs, start, stop):
    nc.tensor.matmul(out, lhsT=lhsT, rhs=rhs, start=start, stop=stop)


def _bcast(t, n):
    """[P, r] tile -> [P, n, r] AP with a stride-0 middle dim (free-dim
    broadcast; verified to work through walrus on DVE tensor ops)."""
    ap = t[:, :]
    return AP(ap.tensor, ap.offset, [ap.ap[0], [0, n], ap.ap[1]])


def build_nc(b_core=BCORE, r=R, niter_cap=None):
    assert b_core % r == 0
    niter = b_core // r
    if niter_cap is not None:
        niter = min(niter, niter_cap)
    nc = bacc.Bacc()

    qT = nc.dram_tensor("qT", [DIM, b_core], BF16, kind="ExternalInput")
    kT = [
        nc.dram_tensor(f"k{s}T", [DIM, b_core], BF16, kind="ExternalInput")
        for s in range(S)
    ]
    wT = {
        n: nc.dram_tensor(n, [DIM, DIM], BF16, kind="ExternalInput")
        for n in ("wqT", "wkT")
    }
    wpT = [
        nc.dram_tensor(f"wp{h}T", [DIM, DIM], BF16, kind="ExternalInput")
        for h in range(H)
    ]
    bias_d = {
        n: nc.dram_tensor(n, [DIM], F32, kind="ExternalInput")
        for n in ("bq", "bk", "bout")
    }
    gamma_d = nc.dram_tensor("gamma", [DIM], F32, kind="ExternalInput")
    beta_d = nc.dram_tensor("beta", [DIM], F32, kind="ExternalInput")
    selscore_d = nc.dram_tensor("selscore", [P, S * H, S * H], BF16, kind="ExternalInput")
    sel3_d = nc.dram_tensor("sel3", [S * H, H], BF16, kind="ExternalInput")
    sel9_d = nc.dram_tensor("sel9", [H, S * H], BF16, kind="ExternalInput")
    rowsel9_d = nc.dram_tensor("rowsel9", [S * H, S * H, P], BF16, kind="ExternalInput")
    ones128_d = nc.dram_tensor("ones128", [P, 1], BF16, kind="ExternalInput")
    ones1_d = nc.dram_tensor("ones1", [1, P], F32R, kind="ExternalInput")
    out_d = nc.dram_tensor("out", [DIM, b_core], BF16, kind="ExternalOutput")

    qT_t = qT[:].rearrange("(ko p) n -> p ko n", p=P)
    kT_t = [k[:].rearrange("(ko p) n -> p ko n", p=P) for k in kT]
    out_t = out_d[:].rearrange("(ko p) n -> p ko n", p=P)

    with tile.TileContext(nc) as tc, ExitStack() as ctx:
        ctx.enter_context(nc.allow_low_precision(reason="bf16 matmul pipeline"))
        wpool = ctx.enter_context(tc.tile_pool(name="wpool", bufs=1))
        qpool = ctx.enter_context(tc.tile_pool(name="qpool", bufs=3))
        kpool = ctx.enter_context(tc.tile_pool(name="kpool", bufs=5))
        dlpool = ctx.enter_context(tc.tile_pool(name="dlpool", bufs=2))
        qcpool = ctx.enter_context(tc.tile_pool(name="qcpool", bufs=1))
        kcpool = ctx.enter_context(tc.tile_pool(name="kcpool", bufs=6))
        ktpool = ctx.enter_context(tc.tile_pool(name="ktpool", bufs=5))
        abpool = ctx.enter_context(tc.tile_pool(name="abpool", bufs=4))
        xpool = ctx.enter_context(tc.tile_pool(name="xpool", bufs=1))
        xsqpool = ctx.enter_context(tc.tile_pool(name="xsqpool", bufs=2))
        wtpool = ctx.enter_context(tc.tile_pool(name="wtpool", bufs=1))
        xtpool = ctx.enter_context(tc.tile_pool(name="xtpool", bufs=1))
        ypool = ctx.enter_context(tc.tile_pool(name="ypool", bufs=2))
        tbpool = ctx.enter_context(tc.tile_pool(name="tbpool", bufs=2))
        smpool = ctx.enter_context(tc.tile_pool(name="smpool", bufs=2))
        psmm = ctx.enter_context(tc.tile_pool(name="psmm", bufs=4, space="PSUM"))
        psbc = ctx.enter_context(tc.tile_pool(name="psbc", bufs=2, space="PSUM"))
        pssm = ctx.enter_context(tc.tile_pool(name="pssm", bufs=2, space="PSUM"))

        # ---- resident constants ----
        w_sb = {}
        for n in wT:
            w = wpool.tile([P, KO, KO, P], BF16, name=f"w_{n}")
            nc.sync.dma_start(
                out=w,
                in_=wT[n][:].rearrange("(ko p) (jo m) -> p ko jo m", p=P, m=P),
            )
            w_sb[n] = w
        wp_sb = []
        for h in range(H):
            w = wpool.tile([P, KO, KO, P], BF16, name=f"w_wp{h}")
            nc.sync.dma_start(
                out=w,
                in_=wpT[h][:].rearrange("(ko p) (jo m) -> p ko jo m", p=P, m=P),
            )
            wp_sb.append(w)
        bias_sb = {}
        for n in bias_d:
            t = wpool.tile([P, KO], F32, name=f"b_{n}")
            nc.sync.dma_start(out=t, in_=bias_d[n][:].rearrange("(jo m) -> m jo", m=P))
            bias_sb[n] = t
        beta_col = wpool.tile([P, KO], F32, name="beta_col")
        nc.sync.dma_start(out=beta_col, in_=beta_d[:].rearrange("(jo m) -> m jo", m=P))
        gamma_col = wpool.tile([P, KO], F32, name="gamma_col")
        nc.sync.dma_start(out=gamma_col, in_=gamma_d[:].rearrange("(jo m) -> m jo", m=P))
        ones1 = wpool.tile([1, P], F32R, name="ones1")
        nc.sync.dma_start(out=ones1, in_=ones1_d[:])
        selscore_sb = wpool.tile([P, S * H, S * H], BF16, name="selscore_sb")
        nc.sync.dma_start(out=selscore_sb, in_=selscore_d[:])
        sel3_sb = wpool.tile([S * H, H], BF16, name="sel3_sb")
        nc.sync.dma_start(out=sel3_sb, in_=sel3_d[:])
        sel9_sb = wpool.tile([H, S * H], BF16, name="sel9_sb")
        nc.sync.dma_start(out=sel9_sb, in_=sel9_d[:])
        rowsel9_sb = wpool.tile([S * H, S * H, P], BF16, name="rowsel9_sb")
        nc.sync.dma_start(out=rowsel9_sb, in_=rowsel9_d[:])
        ones128 = wpool.tile([P, 1], BF16, name="ones128")
        nc.sync.dma_start(out=ones128, in_=ones128_d[:])

        def stage_b(prev, mid_cb=None):
            """Fused out-projection + residual + LayerNorm + store for the
            previous iteration; emitted mid-stream of the current one so the
            PE has independent matmul work while this iteration's softmax ->
            key-mix DVE chain drains."""
            kt, q_prev, n0p = prev
            x_sb = xpool.tile([P, KO, r], BF16, name="x_sb", tag="x")
            xt = xtpool.tile([P, KO, r], BF16, name="xt", tag="xt")
            for jo in range(KO):
                op_ps = psmm.tile([P, r], F32, name="op_ps", tag="mm")
                for h in range(H):
                    for ko in range(KO):
                        _mm(nc, op_ps, wp_sb[h][:, ko, jo, :], kt[h][:, ko, :],
                            h == 0 and ko == 0, h == H - 1 and ko == KO - 1)
                nc.scalar.activation(
                    out=xt[:, jo, :], in_=op_ps, func=AF.Identity,
                    bias=bias_sb["bout"][:, jo : jo + 1],
                )
                if jo == 0 and mid_cb is not None:
                    # attn broadcasts + bf16 copies land here so the ACT
                    # copies (and the key mix behind them) start before
                    # stage_b's xt/xsq ACT backlog
                    mid_cb()
            # wide residual add (bf16, 4x): x = (k~ proj + b_out) + query
            nc.vector.tensor_add(out=x_sb, in0=xt, in1=q_prev)

            # ---- LayerNorm statistics (partition sums via ones-matmul) ----
            sx_ps = pssm.tile([1, r], F32, name="sx_ps", tag="pssm")
            for jo in range(KO):
                _mm(nc, sx_ps, ones128[:], x_sb[:, jo, :], jo == 0, jo == KO - 1)
            sxx_ps = pssm.tile([1, r], F32, name="sxx_ps", tag="pssm")
            for jo in range(KO):
                xsq = xsqpool.tile([P, r], BF16, name="xsq", tag="xsq")
                nc.scalar.activation(out=xsq, in_=x_sb[:, jo, :], func=AF.Square)
                _mm(nc, sxx_ps, ones128[:], xsq[:], jo == 0, jo == KO - 1)

            mv0 = smpool.tile([1, r], F32, name="mv0", tag="mv0", bufs=1)
            nc.vector.tensor_scalar_mul(out=mv0, in0=sx_ps, scalar1=1.0 / DIM)
            mv1 = smpool.tile([1, r], F32, name="mv1", tag="mv1", bufs=1)
            nc.vector.tensor_scalar_mul(out=mv1, in0=sxx_ps, scalar1=1.0 / DIM)
            # muvar holds mu^2 first, then var+eps (in place)
            muvar = smpool.tile([1, r], F32, name="muvar", tag="muvar", bufs=1)
            nc.vector.tensor_mul(out=muvar, in0=mv0, in1=mv0)
            nc.vector.scalar_tensor_tensor(
                out=muvar, in0=mv1, scalar=EPS, in1=muvar, op0=OP.add, op1=OP.subtract
            )
            # rstd = rsqrt(var) via the int32 magic-constant seed + 2 Newton steps
            yi = smpool.tile([1, r], I32, name="yi", tag="yi", bufs=1)
            nc.vector.tensor_scalar(
                out=yi, in0=muvar.bitcast(I32), scalar1=1, scalar2=None,
                op0=OP.arith_shift_right,
            )
            nc.vector.tensor_scalar(
                out=yi, in0=yi, scalar1=-1, scalar2=MAGIC,
                op0=OP.mult, op1=OP.add,
            )
            yv = yi.bitcast(F32)
            t_w = smpool.tile([1, r], F32, name="t_w", tag="t_w", bufs=1)
            rstd = smpool.tile([1, r], F32, name="rstd", tag="rstd", bufs=1)
            # single Newton step: magic seed is within 3.5%; one step lands
            # at ~1.8e-3 relative rstd error -- enough for the 2e-2 gate
            nc.vector.tensor_mul(out=t_w, in0=yv, in1=yv)
            nc.vector.scalar_tensor_tensor(
                out=t_w, in0=t_w, scalar=-0.5, in1=muvar,
                op0=OP.mult, op1=OP.mult,
            )
            nc.vector.tensor_scalar_add(out=t_w, in0=t_w, scalar1=1.5)
            nc.vector.tensor_mul(out=rstd, in0=yv, in1=t_w)
            m2 = smpool.tile([1, r], F32, name="m2", tag="m2", bufs=1)
            nc.vector.tensor_mul(out=m2, in0=mv0, in1=rstd)

            # broadcast rstd/mu*rstd to 128 partitions on the otherwise-idle
            # GpSimd engine (tiny ACT bf16 rounding first); frees 2 PE
            # matmuls + 2 wide ACT copies per tile
            rstd_b1 = smpool.tile([1, r], BF16, name="rstd_b1", tag="r1", bufs=1)
            nc.scalar.activation(out=rstd_b1, in_=rstd, func=AF.Identity)
            rstd_bf = tbpool.tile([P, r], BF16, name="rstd_bf", tag="tb")
            nc.gpsimd.partition_broadcast(rstd_bf[:, :], rstd_b1[:, :])
            m2_b1 = smpool.tile([1, r], BF16, name="m2_b1", tag="m1", bufs=1)
            nc.scalar.activation(out=m2_b1, in_=m2, func=AF.Identity)
            m2_bf = tbpool.tile([P, r], BF16, name="m2_bf", tag="tb")
            nc.gpsimd.partition_broadcast(m2_bf[:, :], m2_b1[:, :])

            # wide normalize: u = x * rstd - mu*rstd (two [P, KO*r] 4x ops),
            # then per-chunk gamma/beta tensor_scalar (per-partition scalars)
            u_w = wtpool.tile([P, KO, r], BF16, name="u_w", tag="wt")
            nc.vector.tensor_mul(out=u_w, in0=x_sb, in1=_bcast(rstd_bf, KO))
            nc.vector.tensor_sub(out=u_w, in0=u_w, in1=_bcast(m2_bf, KO))
            for jo in range(KO):
                y = ypool.tile([P, r], BF16, name="y", tag="y")
                nc.vector.tensor_scalar(
                    out=y, in0=u_w[:, jo, :], scalar1=gamma_col[:, jo : jo + 1],
                    scalar2=beta_col[:, jo : jo + 1], op0=OP.mult, op1=OP.add,
                )
                nc.sync.dma_start(out=out_t[:, jo, bass.ds(n0p, r)], in_=y)

        def emit_iter(n0, prev, last=False):
            # ---- input DMAs (prefetch under previous iteration's compute) ----
            q_in = qpool.tile([P, KO, r], BF16, name="q_in", tag="q_in")
            nc.sync.dma_start(out=q_in, in_=qT_t[:, :, bass.ds(n0, r)])
            k_in = []
            for s in range(S):
                kt_ = kpool.tile([P, KO, r], BF16, name=f"k_in{s}", tag="k_in")
                nc.sync.dma_start(out=kt_, in_=kT_t[s][:, :, bass.ds(n0, r)])
                k_in.append(kt_)
            # key deltas for the 2-term key mix (sum_s attn = 1):
            #   k~_h = k2 + a_{h,0} (k0-k2) + a_{h,1} (k1-k2)
            dl = []
            for s in range(2):
                d = dlpool.tile([P, KO, r], BF16, name=f"dl{s}", tag="dl")
                nc.vector.tensor_sub(out=d, in0=k_in[s], in1=k_in[2])
                dl.append(d)

            # ---- Q projection into one contiguous [P, KO, r] tile ----
            qc = qcpool.tile([P, KO, r], BF16, name="qc", tag="qc")
            for jo in range(KO):
                qp = psmm.tile([P, r], F32, name="qp", tag="mm")
                for ko in range(KO):
                    _mm(nc, qp, w_sb["wqT"][:, ko, jo, :], q_in[:, ko, :],
                        ko == 0, ko == KO - 1)
                nc.scalar.activation(
                    out=qc[:, jo, :], in_=qp, func=AF.Identity,
                    bias=bias_sb["bq"][:, jo : jo + 1],
                )

            # ---- K projections fused with score accumulation: per (h, s)
            # group, the two head-chunk products run as one wide DVE mul
            # (bf16 4x) and pair-sum in place; the selection matmul is
            # delayed 4 groups so the PE never waits on the previous
            # iteration's key-mix draining ahead of it in the DVE queue ----
            scores_ps = pssm.tile([S * H, r], F32, name="scores_ps", tag="pssm")
            first_score = True
            n_sel = 0
            n_grp = 0
            pending_sel = []

            def flush_sel(force=False):
                nonlocal first_score, n_sel
                while pending_sel and (force or pending_sel[0][0] <= n_grp - 5):
                    _, c, sc = pending_sel.pop(0)
                    _mm(nc, scores_ps, selscore_sb[:, c, :], sc,
                        first_score, n_sel == H * S - 1)
                    first_score = False
                    n_sel += 1

            for h in range(H):
                # the three keys' PSUM chains interleave ko-major under a
                # shared lhsT so each wkT weight tile is streamed once per
                # head chunk instead of once per key (amortizes PE weight
                # loads)
                kcs = [
                    kcpool.tile([P, 2, r], BF16, name="kc", tag="kc")
                    for _ in range(S)
                ]
                for j, jo in enumerate((2 * h, 2 * h + 1)):
                    kps = [
                        psmm.tile([P, r], F32, name="kp", tag="mm")
                        for _ in range(S)
                    ]
                    for ko in range(KO):
                        for s in range(S):
                            _mm(nc, kps[s], w_sb["wkT"][:, ko, jo, :],
                                k_in[s][:, ko, :], ko == 0, ko == KO - 1)
                    for s in range(S):
                        nc.scalar.activation(
                            out=kcs[s][:, j, :], in_=kps[s], func=AF.Identity,
                            bias=bias_sb["bk"][:, jo : jo + 1],
                        )
                for s in range(S):
                    kc = kcs[s]
                    # wide product over both head chunks, then in-place pair sum
                    nc.vector.tensor_mul(
                        out=kc, in0=kc, in1=qc[:, 2 * h : 2 * h + 2, :]
                    )
                    nc.vector.tensor_add(
                        out=kc[:, 1, :], in0=kc[:, 0, :], in1=kc[:, 1, :]
                    )
                    pending_sel.append((n_grp, s * H + h, kc[:, 1, :]))
                    n_grp += 1
                    flush_sel()
            flush_sel(force=True)

            # ---- softmax over S=3 (no max subtraction; |scores/16| < ~8) ----
            exp_sb = smpool.tile([S * H, r], BF16, name="exp_sb", tag="sm9")
            nc.scalar.activation(
                out=exp_sb, in_=scores_ps, func=AF.Exp, scale=1.0 / 16.0
            )
            den_ps = pssm.tile([H, r], F32, name="den_ps", tag="pssm")
            _mm(nc, den_ps, sel3_sb[:], exp_sb[:], True, True)
            den_r = smpool.tile([H, r], F32, name="den_r", tag="sm3", bufs=1)
            nc.vector.reciprocal_approx_fast(out=den_r, in_=den_ps)
            den_rb = smpool.tile([H, r], BF16, name="den_rb", tag="sm3b", bufs=1)
            nc.scalar.activation(out=den_rb, in_=den_r, func=AF.Identity)
            denb_ps = pssm.tile([S * H, r], F32, name="denb_ps", tag="pssm")
            _mm(nc, denb_ps, sel9_sb[:], den_rb[:], True, True)
            denb_bf = smpool.tile([S * H, r], BF16, name="denb_bf", tag="sm9b", bufs=1)
            nc.scalar.activation(out=denb_bf, in_=denb_ps, func=AF.Identity)
            attn_sb = smpool.tile([S * H, r], BF16, name="attn_sb", tag="sm9")
            nc.vector.tensor_mul(out=attn_sb, in0=exp_sb, in1=denb_bf)

            # ---- attn row broadcasts (PE rank-1) -> SBUF bf16; only s=0,1
            # are needed thanks to the delta form of the key mix ----
            ab_bf = {}
            kt = [
                ktpool.tile([P, KO, r], BF16, name=f"kt{h}", tag="kt")
                for h in range(H)
            ]

            def mix_head(h):
                kth = kt[h]
                m = wtpool.tile([P, KO, r], BF16, name="m", tag="wt")
                nc.vector.tensor_mul(out=m, in0=dl[0], in1=_bcast(ab_bf[(h, 0)], KO))
                nc.vector.tensor_mul(out=kth, in0=dl[1], in1=_bcast(ab_bf[(h, 1)], KO))
                nc.vector.tensor_add(out=kth, in0=kth, in1=m)
                nc.vector.tensor_add(out=kth, in0=kth, in1=k_in[2])

            def emit_ab():
                for h in range(H):
                    for s in range(2):
                        ab_ps = psbc.tile([P, r], F32, name="ab_ps", tag="bc")
                        _mm(nc, ab_ps, rowsel9_sb[:, s * H + h, :], attn_sb[:],
                            True, True)
                        t = abpool.tile([P, r], BF16, name="ab_bf", tag="ab")
                        nc.scalar.activation(out=t, in_=ab_ps, func=AF.Identity)
                        ab_bf[(h, s)] = t
                if last:
                    # no next iteration to hide the key mix under: emit it
                    # here so the drain stage_b's matmuls start ~15us earlier
                    for h in range(H):
                        mix_head(h)

            # ---- previous iteration's fused out-proj + LayerNorm lands here:
            # its 122 PE matmuls cover this iteration's softmax/key-mix
            # ACT/DVE latency; the attn broadcasts are emitted after its
            # first projection chain via mid_cb ----
            if prev is not None:
                stage_b(prev, mid_cb=emit_ab)
            else:
                emit_ab()

            # ---- per-head key mix on DVE: 4 wide [P, KO*r] bf16 ops per
            # head with the attn rows broadcast along KO via stride-0 APs;
            # consumed by the fused out-projection of the NEXT iteration ----
            if not last:
                for h in range(H):
                    mix_head(h)

            return (kt, q_in, n0)

        prev = None
        for i in range(niter):
            prev = emit_iter(i * r, prev, last=(i == niter - 1))
        stage_b(prev)

    nc.compile()
    return nc


def make_consts():
    sh = S * H
    f = np.float32
    selscore = np.broadcast_to(np.eye(sh, dtype=f)[None], (P, sh, sh))
    selscore = np.ascontiguousarray(selscore)
    k = np.arange(sh)
    sel3 = (k[:, None] % H == np.arange(H)[None, :]).astype(f)
    sel9 = (np.arange(sh)[None, :] % H == np.arange(H)[:, None]).astype(f)
    rowsel9 = np.ascontiguousarray(
        np.broadcast_to(np.eye(sh, dtype=f)[:, :, None], (sh, sh, P))
    )
    return selscore, sel3, sel9, rowsel9


def _bf16(a):
    import ml_dtypes

    return np.asarray(a, np.float32).astype(ml_dtypes.bfloat16)


def make_in_maps(inputs, b_core=BCORE, ncores=NCORES):
    f = np.float32
    q = np.asarray(inputs["query"], f)
    keys = [np.asarray(inputs[f"key{s}"], f) for s in range(S)]
    Wv = np.asarray(inputs["Wv"], f)
    Wo = np.asarray(inputs["Wo"], f)
    bv = np.asarray(inputs["bv"], f)
    bo = np.asarray(inputs["bo"], f)
    shared = {
        "wqT": _bf16(np.asarray(inputs["Wq"], f).T),
        "wkT": _bf16(np.asarray(inputs["Wk"], f).T),
        "bq": np.asarray(inputs["bq"], f),
        "bk": np.asarray(inputs["bk"], f),
        "bout": bo + Wo @ bv,
        "gamma": np.asarray(inputs["gamma"], f),
        "beta": np.asarray(inputs["beta"], f),
    }
    for h in range(H):
        Wv_h = Wv[h * HD : (h + 1) * HD, :]
        Wo_h = Wo[:, h * HD : (h + 1) * HD]
        shared[f"wp{h}T"] = _bf16(Wv_h.T @ Wo_h.T)
    selscore, sel3, sel9, rowsel9 = make_consts()
    shared.update({
        "selscore": _bf16(selscore), "sel3": _bf16(sel3), "sel9": _bf16(sel9),
        "rowsel9": _bf16(rowsel9),
        "ones128": _bf16(np.ones((P, 1), f)), "ones1": np.ones((1, P), f),
    })
    in_maps = []
    for c in range(ncores):
        sl = slice(c * b_core, (c + 1) * b_core)
        m = dict(shared)
        m["qT"] = _bf16(q[sl].T)
        for s in range(S):
            m[f"k{s}T"] = _bf16(keys[s][sl].T)
        in_maps.append(m)
    return in_maps


_NC_CACHE = {}


def _get_nc(b_core=BCORE, r=R, niter_cap=None):
    key = (b_core, r, niter_cap)
    if key not in _NC_CACHE:
        _NC_CACHE[key] = build_nc(b_core, r, niter_cap)
    return _NC_CACHE[key]


def run(inputs):
    """Run on 8 NeuronCores; returns the full output."""
    nc = _get_nc()
    in_maps = make_in_maps(inputs)
    res = run_bass_kernel_spmd(nc, in_maps, core_ids=list(range(NCORES)), trace=False)
    y = np.empty((B, DIM), np.float32)
    for c in range(NCORES):
        y[c * BCORE : (c + 1) * BCORE] = res.results[c]["out"].T.astype(np.float32)
    return y


def kernel(**inputs):
    return run(inputs)
